# revision 2
# baseline (speedup 1.0000x reference)
"""Multi-head attention (B=2,S=2048,D=1024,H=16,HD=64) with RoPE on 8 TRN2 cores.

Sharding: core c handles batch b=c//4 and head-group hg=c%4 (4 heads = 256
output cols). Each (b, head) is independent -> no collectives.

Per-core on-chip layout (all matmul inputs bf16, PSUM f32):
  xT [1024,2048]    (D on partitions; host passes x[b].T)
  Q^T,K^T [256,2048] via matmul(lhsT=W^T tile, rhs=xT tile); bias folded in as
                     a K=1 matmul with a ones row; RoPE applied in-place via a
                     pair-swap permutation matmul + cos/sin elementwise (DVE)
  V [2048,4*65]     natural layout + a ones column per head (gives softmax
                     denominators for free during the AV matmul)
  scores^T tiles [128tk, 512q] per head = matmul(lhsT=K^T slice, rhs=Q^T slice)
                     two heads packed in the PE array via tile_position rows
  P^T = exp(scores^T * 0.125) on ACT (no max-subtraction needed: |s|<~6)
  out^T[65,512] accum = matmul(lhsT=[V_h|1], rhs=P^T)  (row 64 = denominators)
  PE-transpose out^T -> [128q, 65], multiply by 1/denominator (DVE), store f32.
"""

import numpy as np

B, S, D, H, HD = 2, 2048, 1024, 16, 64
NCORES = 8
CPC = 256  # output cols per core (4 heads)

_PROG = {}


def _rope_tables():
    i = np.arange(HD // 2, dtype=np.float64)
    theta = 10000.0 ** (-2.0 * i / HD)
    t = np.arange(S, dtype=np.float64)
    ang = np.outer(theta, t)  # [32, S]
    rowi = (np.arange(128) % 64) // 2
    cosf = np.cos(ang)[rowi].astype(np.float32)  # [128, S]
    sinf = np.sin(ang)[rowi].astype(np.float32)
    return cosf, sinf


def _perm_matrix():
    # permT[p, m]: rot[m] = sum_p permT[p, m] * q[p]
    # rot[2i] = -q[2i+1], rot[2i+1] = +q[2i]
    p = np.zeros((128, 128), dtype=np.float32)
    for i in range(64):
        p[2 * i + 1, 2 * i] = -1.0
        p[2 * i, 2 * i + 1] = 1.0
    return p


def _build_program():
    import concourse.bacc as bacc
    import concourse.mybir as mybir
    from concourse import tile

    f32 = mybir.dt.float32
    bf16 = mybir.dt.bfloat16
    mult = mybir.AluOpType.mult
    Exp = mybir.ActivationFunctionType.Exp

    nc = bacc.Bacc(None)

    xt_d = nc.declare_dram_parameter("xt", [D, S], f32, isOutput=False)
    wq_d = nc.declare_dram_parameter("wqt", [D, CPC], f32, isOutput=False)
    wk_d = nc.declare_dram_parameter("wkt", [D, CPC], f32, isOutput=False)
    wv_d = nc.declare_dram_parameter("wvt", [D, CPC], f32, isOutput=False)
    bq_d = nc.declare_dram_parameter("bq", [1, CPC], f32, isOutput=False)
    bk_d = nc.declare_dram_parameter("bk", [1, CPC], f32, isOutput=False)
    bv_d = nc.declare_dram_parameter("bv", [1, CPC], f32, isOutput=False)
    cos_d = nc.declare_dram_parameter("cosf", [128, S], f32, isOutput=False)
    sin_d = nc.declare_dram_parameter("sinf", [128, S], f32, isOutput=False)
    perm_d = nc.declare_dram_parameter("permT", [128, 128], f32, isOutput=False)
    id_d = nc.declare_dram_parameter("ident", [128, 128], f32, isOutput=False)
    out_d = nc.declare_dram_parameter("out", [S, CPC], f32, isOutput=True)

    NT = S // 128   # 16 token tiles
    NQC = S // 512  # 4 query chunks
    NKT = D // 128  # 8 contraction tiles

    with tile.TileContext(nc) as tc:
        with tc.tile_pool(name="persist", bufs=1) as pp:
            # persistent SBUF tensors
            xt_sb = [pp.tile([128, NKT, 512], bf16, tag=f"xt{q}", name=f"xt{q}") for q in range(NQC)]
            w_sb = [pp.tile([128, NKT, CPC], bf16, tag=f"w{j}", name=f"w{j}") for j in range(3)]
            b_sb = [pp.tile([1, CPC], bf16, tag=f"b{j}", name=f"b{j}") for j in range(3)]
            cos_sb = pp.tile([128, S], bf16, tag="cos", name="cos")
            sin_sb = pp.tile([128, S], bf16, tag="sin", name="sin")
            perm_sb = pp.tile([128, 128], bf16, tag="perm", name="perm")
            id_sb = pp.tile([128, 128], f32, tag="ident", name="ident")
            ones_sb = pp.tile([1, 512], bf16, tag="ones", name="ones")
            qf = [pp.tile([128, S], bf16, tag=f"qf{c}", name=f"qf{c}") for c in range(2)]
            kf = [pp.tile([128, S], bf16, tag=f"kf{c}", name=f"kf{c}") for c in range(2)]
            v_sb = [pp.tile([128, 4, HD + 1], bf16, tag=f"v{t}", name=f"v{t}") for t in range(NT)]

            # ---- loads (SWDGE casts f32 -> bf16 in flight) ----
            for j, wd in enumerate((wq_d, wk_d, wv_d)):
                nc.gpsimd.dma_start(
                    w_sb[j][:], wd.rearrange("(a p) c -> p a c", p=128)
                )
            for j, bd in enumerate((bq_d, bk_d, bv_d)):
                nc.gpsimd.dma_start(b_sb[j][:], bd[:])
            nc.gpsimd.dma_start(cos_sb[:], cos_d[:])
            nc.gpsimd.dma_start(sin_sb[:], sin_d[:])
            nc.gpsimd.dma_start(perm_sb[:], perm_d[:])
            nc.sync.dma_start(id_sb[:], id_d[:])
            xt_r = xt_d.rearrange("(a p) t -> p a t", p=128)
            for q in range(NQC):
                nc.gpsimd.dma_start(xt_sb[q][:], xt_r[:, :, q * 512:(q + 1) * 512])
            nc.vector.memset(ones_sb[:], 1.0)

            # ---- phase A: projections + RoPE ----
            with (
                tc.tile_pool(name="psA", bufs=4, space="PSUM") as psA,
                tc.tile_pool(name="stA", bufs=3) as stA,
            ):
                for q in range(NQC):
                    qsl = slice(q * 512, (q + 1) * 512)
                    for ct in range(2):
                        csl = slice(ct * 128, (ct + 1) * 128)
                        for widx, dst in ((0, qf[ct]), (1, kf[ct])):
                            ps = psA.tile([128, 512], f32, tag="ps", name="ps")
                            for kt in range(NKT):
                                nc.tensor.matmul(
                                    ps[:], w_sb[widx][:, kt, csl],
                                    xt_sb[q][:, kt, :],
                                    start=(kt == 0), stop=False,
                                )
                            nc.tensor.matmul(
                                ps[:], b_sb[widx][0:1, csl], ones_sb[:],
                                start=False, stop=True,
                            )
                            qraw = stA.tile([128, 512], bf16, tag="qraw", name="qraw")
                            nc.vector.tensor_copy(qraw[:], ps[:])
                            rps = psA.tile([128, 512], f32, tag="ps", name="ps")
                            nc.tensor.matmul(
                                rps[:], perm_sb[:], qraw[:], start=True, stop=True
                            )
                            rsb = stA.tile([128, 512], bf16, tag="rsb", name="rsb")
                            nc.vector.tensor_copy(rsb[:], rps[:])
                            t1 = stA.tile([128, 512], bf16, tag="t1", name="t1")
                            nc.vector.tensor_mul(t1[:], qraw[:], cos_sb[:, qsl])
                            t2 = stA.tile([128, 512], bf16, tag="t2", name="t2")
                            nc.vector.tensor_mul(t2[:], rsb[:], sin_sb[:, qsl])
                            nc.vector.tensor_add(dst[:, qsl], t1[:], t2[:])
                    for ti in range(4):
                        tt = q * 4 + ti
                        ps = psA.tile([128, CPC], f32, tag="ps", name="ps")
                        for kt in range(NKT):
                            nc.tensor.matmul(
                                ps[:], xt_sb[q][:, kt, ti * 128:(ti + 1) * 128],
                                w_sb[2][:, kt, :],
                                start=(kt == 0), stop=False,
                            )
                        nc.tensor.matmul(
                            ps[:], ones_sb[0:1, 0:128], b_sb[2][:],
                            start=False, stop=True,
                        )
                        nc.vector.tensor_copy(
                            v_sb[tt][:, :, 0:HD],
                            ps[:].rearrange("p (h d) -> p h d", h=4),
                        )
                        nc.vector.memset(v_sb[tt][:, :, HD:HD + 1], 1.0)

            # ---- phase B: attention ----
            with (
                tc.tile_pool(name="psS", bufs=2, space="PSUM") as psS,
                tc.tile_pool(name="psAV", bufs=2, space="PSUM") as psAV,
                tc.tile_pool(name="psT", bufs=2, space="PSUM") as psT,
                tc.tile_pool(name="ptp", bufs=4) as ptp,
                tc.tile_pool(name="avs", bufs=3) as avsp,
                tc.tile_pool(name="rsp", bufs=4) as rsp,
                tc.tile_pool(name="outp", bufs=2) as outp,
            ):
                for q in range(NQC):
                    qsl = slice(q * 512, (q + 1) * 512)
                    outs = [outp.tile([128, CPC], f32, tag=f"o{j}", name=f"o{j}") for j in range(4)]
                    for pr in range(2):
                        av_e = psAV.tile([HD + 1, 512], f32, tag="av", name="av")
                        av_o = psAV.tile([HD + 1, 512], f32, tag="av", name="av")
                        for tk in range(NT):
                            tsl = slice(tk * 128, (tk + 1) * 128)
                            sc = psS.tile([128, 1024], f32, tag="sc", name="sc")
                            nc.tensor.matmul(
                                sc[:, 0:512], kf[pr][0:64, tsl], qf[pr][0:64, qsl],
                                start=True, stop=True, tile_position=(0, 0),
                            )
                            nc.tensor.matmul(
                                sc[:, 512:1024], kf[pr][64:128, tsl],
                                qf[pr][64:128, qsl],
                                start=True, stop=True, tile_position=(64, 0),
                            )
                            pt = ptp.tile([128, 1024], bf16, tag="pt", name="pt")
                            nc.scalar.activation(pt[:], sc[:], Exp, scale=0.125)
                            nc.tensor.matmul(
                                av_e[:], v_sb[tk][:, 2 * pr, :], pt[:, 0:512],
                                start=(tk == 0), stop=(tk == NT - 1),
                                skip_group_check=True,
                            )
                            nc.tensor.matmul(
                                av_o[:], v_sb[tk][:, 2 * pr + 1, :], pt[:, 512:1024],
                                start=(tk == 0), stop=(tk == NT - 1),
                                skip_group_check=True,
                            )
                        for e, av in ((0, av_e), (1, av_o)):
                            hc = 2 * pr + e
                            avs = avsp.tile([HD + 1, 512], f32, tag="avs", name="avs")
                            nc.vector.tensor_copy(avs[:], av[:])
                            for j in range(4):
                                tp = psT.tile([128, HD + 1], f32, tag="tp", name="tp")
                                nc.tensor.transpose(
                                    tp[:], avs[:, j * 128:(j + 1) * 128],
                                    id_sb[0:HD + 1, 0:HD + 1],
                                )
                                rs = rsp.tile([128, 1], f32, tag="rs", name="rs")
                                nc.vector.reciprocal(rs[:], tp[:, HD:HD + 1])
                                nc.vector.tensor_scalar_mul(
                                    outs[j][:, hc * HD:(hc + 1) * HD],
                                    tp[:, 0:HD], rs[:],
                                )
                    for j in range(4):
                        r0 = (q * 4 + j) * 128
                        nc.sync.dma_start(out_d[r0:r0 + 128, :], outs[j][:])

    nc.compile()
    return nc


def _get_program():
    if "nc" not in _PROG:
        _PROG["nc"] = _build_program()
    return _PROG["nc"]


def _in_maps(x, wq_w, wq_b, wk_w, wk_b, wv_w, wv_b):
    cosf, sinf = _rope_tables()
    permT = _perm_matrix()
    ident = np.eye(128, dtype=np.float32)
    maps = []
    for c in range(NCORES):
        b, hg = divmod(c, 4)
        sl = slice(hg * CPC, (hg + 1) * CPC)
        maps.append({
            "xt": np.ascontiguousarray(x[b].T.astype(np.float32)),
            "wqt": np.ascontiguousarray(wq_w[sl].T.astype(np.float32)),
            "wkt": np.ascontiguousarray(wk_w[sl].T.astype(np.float32)),
            "wvt": np.ascontiguousarray(wv_w[sl].T.astype(np.float32)),
            "bq": wq_b[sl].reshape(1, CPC).astype(np.float32),
            "bk": wk_b[sl].reshape(1, CPC).astype(np.float32),
            "bv": wv_b[sl].reshape(1, CPC).astype(np.float32),
            "cosf": cosf, "sinf": sinf, "permT": permT, "ident": ident,
        })
    return maps


def _gather(results):
    out = np.empty((B, S, D), dtype=np.float32)
    for c in range(NCORES):
        b, hg = divmod(c, 4)
        out[b, :, hg * CPC:(hg + 1) * CPC] = results[c]["out"]
    return out


def kernel(x, wq_w, wq_b, wk_w, wk_b, wv_w, wv_b):
    from concourse.bass_utils import run_bass_kernel_spmd
    nc = _get_program()
    maps = _in_maps(x, wq_w, wq_b, wk_w, wk_b, wv_w, wv_b)
    res = run_bass_kernel_spmd(nc, maps, core_ids=list(range(NCORES)))
    return _gather(res.results)


def kernel_profiled(x, wq_w, wq_b, wk_w, wk_b, wv_w, wv_b):
    """Same as kernel() but requests an NTFF trace; returns (out, results)."""
    from concourse.bass_utils import run_bass_kernel_spmd
    nc = _get_program()
    maps = _in_maps(x, wq_w, wq_b, wk_w, wk_b, wv_w, wv_b)
    res = run_bass_kernel_spmd(
        nc, maps, core_ids=list(range(NCORES)), trace=True
    )
    return _gather(res.results), res


# revision 15
# speedup vs baseline: 1.3569x; 1.3569x over previous
"""Multi-head attention (B=2,S=2048,D=1024,H=16,HD=64) with RoPE on 8 TRN2 cores.

Sharding: core c handles batch b=c//4 and head-group hg=c%4 (4 heads = 256
output cols). Each (b, head) is independent -> no collectives.

Per-core on-chip layout (all matmul inputs bf16, PSUM f32):
  xT [1024,2048]    (D on partitions; host passes x[b].T)
  Q^T,K^T [256,2048] via matmul(lhsT=W^T tile, rhs=xT tile); bias folded in as
                     a K=1 matmul with a ones row; RoPE applied in-place via a
                     pair-swap permutation matmul + cos/sin elementwise (DVE)
  V [2048,4*65]     natural layout + a ones column per head (gives softmax
                     denominators for free during the AV matmul)
  scores^T tiles [128tk, 512q] per head = matmul(lhsT=K^T slice, rhs=Q^T slice)
                     two heads packed in the PE array via tile_position rows
  P^T = exp(scores^T * 0.125) on ACT (no max-subtraction needed: |s|<~6)
  out^T[65,512] accum = matmul(lhsT=[V_h|1], rhs=P^T)  (row 64 = denominators)
  PE-transpose out^T -> [128q, 65], multiply by 1/denominator (DVE), store f32.
"""

import numpy as np

B, S, D, H, HD = 2, 2048, 1024, 16, 64
NCORES = 8
CPC = 256  # output cols per core (4 heads)

_PROG = {}


def _rope_tables():
    i = np.arange(HD // 2, dtype=np.float64)
    theta = 10000.0 ** (-2.0 * i / HD)
    t = np.arange(S, dtype=np.float64)
    ang = np.outer(theta, t)  # [32, S]
    rowi = (np.arange(128) % 64) // 2
    cosf = np.cos(ang)[rowi].astype(np.float32)  # [128, S]
    sinf = np.sin(ang)[rowi].astype(np.float32)
    return cosf, sinf


def _perm_matrix():
    # permT[p, m]: rot[m] = sum_p permT[p, m] * q[p]
    # rot[2i] = -q[2i+1], rot[2i+1] = +q[2i]
    p = np.zeros((128, 128), dtype=np.float32)
    for i in range(64):
        p[2 * i + 1, 2 * i] = -1.0
        p[2 * i, 2 * i + 1] = 1.0
    return p


def _build_program():
    import concourse.bacc as bacc
    import concourse.mybir as mybir
    from concourse import tile

    f32 = mybir.dt.float32
    bf16 = mybir.dt.bfloat16
    mult = mybir.AluOpType.mult
    Exp = mybir.ActivationFunctionType.Exp

    nc = bacc.Bacc(None)

    xt_d = nc.declare_dram_parameter("xt", [D, S], f32, isOutput=False)
    wq_d = nc.declare_dram_parameter("wqt", [D, CPC], f32, isOutput=False)
    wk_d = nc.declare_dram_parameter("wkt", [D, CPC], f32, isOutput=False)
    wv_d = nc.declare_dram_parameter("wvt", [D, CPC], f32, isOutput=False)
    bq_d = nc.declare_dram_parameter("bq", [1, CPC], f32, isOutput=False)
    bk_d = nc.declare_dram_parameter("bk", [1, CPC], f32, isOutput=False)
    bv_d = nc.declare_dram_parameter("bv", [1, CPC], f32, isOutput=False)
    cos_d = nc.declare_dram_parameter("cosf", [128, S], bf16, isOutput=False)
    sin_d = nc.declare_dram_parameter("sinf", [128, S], bf16, isOutput=False)
    perm_d = nc.declare_dram_parameter("permT", [128, 128], bf16, isOutput=False)
    id_d = nc.declare_dram_parameter("ident", [128, 128], f32, isOutput=False)
    out_d = nc.declare_dram_parameter("out", [S, CPC], f32, isOutput=True)

    NT = S // 128   # 16 token tiles
    NQC = S // 512  # 4 query chunks
    NKT = D // 128  # 8 contraction tiles

    with tile.TileContext(nc) as tc:
        with tc.tile_pool(name="persist", bufs=1) as pp:
            # persistent SBUF tensors
            xt_sb = [pp.tile([128, NKT, 512], bf16, tag=f"xt{q}", name=f"xt{q}") for q in range(NQC)]
            w_sb = [pp.tile([128, NKT, CPC], bf16, tag=f"w{j}", name=f"w{j}") for j in range(3)]
            b_sb = [pp.tile([1, CPC], bf16, tag=f"b{j}", name=f"b{j}") for j in range(3)]
            cos_sb = pp.tile([128, S], bf16, tag="cos", name="cos")
            sin_sb = pp.tile([128, S], bf16, tag="sin", name="sin")
            perm_sb = pp.tile([128, 128], bf16, tag="perm", name="perm")
            id_sb = pp.tile([128, 128], f32, tag="ident", name="ident")
            ones_sb = pp.tile([1, 512], bf16, tag="ones", name="ones")
            qf = [pp.tile([128, S], bf16, tag=f"qf{c}", name=f"qf{c}") for c in range(2)]
            kf = [pp.tile([128, S], bf16, tag=f"kf{c}", name=f"kf{c}") for c in range(2)]
            v_sb = [pp.tile([128, 4, HD + 1], bf16, tag=f"v{t}", name=f"v{t}") for t in range(NT)]

            # ---- loads (SWDGE casts f32 -> bf16 in flight) ----
            xt_r = xt_d.rearrange("(a p) t -> p a t", p=128)
            wds = (wq_d, wk_d, wv_d)
            wk_r = wds[1].rearrange("(a p) c -> p a c", p=128)
            nc.gpsimd.dma_start(w_sb[1][:, 0:4, :], wk_r[:, 0:4, :])
            nc.gpsimd.dma_start(
                xt_sb[0][:, 0:4, :], xt_r[:, 0:4, 0:512])
            nc.gpsimd.dma_start(w_sb[1][:, 4:8, :], wk_r[:, 4:8, :])
            nc.gpsimd.dma_start(
                xt_sb[0][:, 4:8, :], xt_r[:, 4:8, 0:512])
            nc.gpsimd.dma_start(w_sb[0][:], wds[0].rearrange("(a p) c -> p a c", p=128))
            for j, bd in enumerate((bq_d, bk_d, bv_d)):
                nc.gpsimd.dma_start(b_sb[j][:], bd[:])
            for h in range(2):
                nc.gpsimd.dma_start(
                    xt_sb[1][:, h * 4:(h + 1) * 4, :],
                    xt_r[:, h * 4:(h + 1) * 4, 512:1024],
                )
            nc.gpsimd.dma_start(w_sb[2][:], wds[2].rearrange("(a p) c -> p a c", p=128))
            for q in range(2, NQC):
                for h in range(2):
                    nc.gpsimd.dma_start(
                        xt_sb[q][:, h * 4:(h + 1) * 4, :],
                        xt_r[:, h * 4:(h + 1) * 4, q * 512:(q + 1) * 512],
                    )
            nc.sync.dma_start(perm_sb[:], perm_d[:])
            nc.sync.dma_start(cos_sb[:], cos_d[:])
            nc.sync.dma_start(sin_sb[:], sin_d[:])
            nc.sync.dma_start(id_sb[:], id_d[:])
            nc.vector.memset(ones_sb[:], 1.0)


            # ---- compute: fully pipelined ----
            # K-proj(pair0) chunks feed score tiles immediately (subtile deps);
            # Q/V/pair-1 projections interleave into the attention stream.
            with (
                tc.tile_pool(name="psS", bufs=2, space="PSUM") as psS,
                tc.tile_pool(name="psX", bufs=4, space="PSUM") as psX,
                tc.tile_pool(name="stA", bufs=3) as stA,
                tc.tile_pool(name="ptp", bufs=28) as ptp,
                tc.tile_pool(name="avs", bufs=3) as avsp,
                tc.tile_pool(name="rsp", bufs=4) as rsp,
                tc.tile_pool(name="outp", bufs=1) as outp,
            ):
                def proj_qk(widx, dst, ct, q):
                    qsl = slice(q * 512, (q + 1) * 512)
                    csl = slice(ct * 128, (ct + 1) * 128)
                    ps = psX.tile([128, 512], f32, tag="av", name="ps")
                    for kt in range(NKT):
                        nc.tensor.matmul(
                            ps[:], w_sb[widx][:, kt, csl], xt_sb[q][:, kt, :],
                            start=(kt == 0), stop=False,
                        )
                    nc.tensor.matmul(
                        ps[:], b_sb[widx][0:1, csl], ones_sb[:],
                        start=False, stop=True,
                    )
                    qraw = stA.tile([128, 512], bf16, tag="qraw", name="qraw")
                    nc.vector.tensor_copy(qraw[:], ps[:])
                    nc.tensor.matmul(ps[:], perm_sb[:], qraw[:], start=True, stop=True)
                    rsb = stA.tile([128, 512], bf16, tag="rsb", name="rsb")
                    nc.vector.tensor_copy(rsb[:], ps[:])
                    t1 = stA.tile([128, 512], bf16, tag="t1", name="t1")
                    nc.vector.tensor_mul(t1[:], qraw[:], cos_sb[:, qsl])
                    t2 = stA.tile([128, 512], bf16, tag="t2", name="t2")
                    nc.vector.tensor_mul(t2[:], rsb[:], sin_sb[:, qsl])
                    nc.vector.tensor_add(dst[:, qsl], t1[:], t2[:])

                def proj_v(q, ti):
                    tt = q * 4 + ti
                    ps = psX.tile([128, CPC], f32, tag="av", name="vps")
                    for kt in range(NKT):
                        nc.tensor.matmul(
                            ps[:], xt_sb[q][:, kt, ti * 128:(ti + 1) * 128],
                            w_sb[2][:, kt, :],
                            start=(kt == 0), stop=False,
                        )
                    nc.tensor.matmul(
                        ps[:], ones_sb[0:1, 0:128], b_sb[2][:],
                        start=False, stop=True,
                    )
                    nc.vector.tensor_copy(
                        v_sb[tt][:, :, 0:HD],
                        ps[:].rearrange("p (h d) -> p h d", h=4),
                    )
                    nc.vector.memset(v_sb[tt][:, :, HD:HD + 1], 1.0)

                out_tiles = {}

                def sc_exp(pr, q, tk):
                    qsl = slice(q * 512, (q + 1) * 512)
                    tsl = slice(tk * 128, (tk + 1) * 128)
                    sc = psS.tile([128, 1024], f32, tag="sc", name="sc")
                    nc.tensor.matmul(
                        sc[:, 0:512], kf[pr][0:64, tsl], qf[pr][0:64, qsl],
                        start=True, stop=True, tile_position=(0, 0),
                    )
                    nc.tensor.matmul(
                        sc[:, 512:1024], kf[pr][64:128, tsl],
                        qf[pr][64:128, qsl],
                        start=True, stop=True, tile_position=(64, 0),
                    )
                    pt = ptp.tile([128, 1024], bf16, tag="pt", name="pt")
                    nc.scalar.activation(pt[:], sc[:], Exp, scale=0.125)
                    return pt

                def av_group(st, g):
                    if g == 0:
                        st["av_e"] = psX.tile([HD + 1, 512], f32, tag="av", name="av_e")
                        st["av_o"] = psX.tile([HD + 1, 512], f32, tag="av", name="av_o")
                    pr = st["pr"]
                    for tk in range(4 * g, 4 * g + 4):
                        nc.tensor.matmul(
                            st["av_e"][:], v_sb[tk][:, 2 * pr, :],
                            st["pts"][tk][:, 0:512],
                            start=(tk == 0), stop=(tk == NT - 1),
                            skip_group_check=True,
                        )
                        nc.tensor.matmul(
                            st["av_o"][:], v_sb[tk][:, 2 * pr + 1, :],
                            st["pts"][tk][:, 512:1024],
                            start=(tk == 0), stop=(tk == NT - 1),
                            skip_group_check=True,
                        )

                def finalize(st):
                    pr, q = st["pr"], st["q"]
                    for e, av in ((0, st["av_e"]), (1, st["av_o"])):
                        hc = 2 * pr + e
                        avs = avsp.tile([HD + 1, 512], f32, tag="avs", name="avs")
                        nc.any.tensor_copy(avs[:], av[:])
                        for j in range(4):
                            qt = q * 4 + j
                            if qt not in out_tiles:
                                out_tiles[qt] = outp.tile(
                                    [128, CPC], f32, tag=f"o{qt}", name=f"o{qt}"
                                )
                            tp = psX.tile([128, HD + 1], f32, tag="av", name="tp")
                            nc.tensor.transpose(
                                tp[:], avs[:, j * 128:(j + 1) * 128],
                                id_sb[0:HD + 1, 0:HD + 1],
                            )
                            rs = rsp.tile([128, 1], f32, tag="rs", name="rs")
                            nc.vector.reciprocal_approx_fast(rs[:], tp[:, HD:HD + 1])
                            nc.vector.tensor_scalar_mul(
                                out_tiles[qt][:, hc * HD:(hc + 1) * HD],
                                tp[:, 0:HD], rs[:],
                            )
                    if pr == 1:
                        for j in range(4):
                            qt = q * 4 + j
                            r0 = qt * 128
                            nc.sync.dma_start(out_d[r0:r0 + 128, :],
                                              out_tiles[qt][:])

                # warmup: PE busy + ACT exp table preload while DMAs land
                wu = stA.tile([128, 512], bf16, tag="qraw", name="wu")
                nc.vector.memset(wu[:], 0.0)
                wups = psX.tile([128, 512], f32, tag="av", name="wups")
                for _ in range(16):
                    nc.tensor.matmul(wups[:], wu[:, 0:128], wu[:],
                                     start=True, stop=True)
                wuexp = stA.tile([128, 512], bf16, tag="rsb", name="wuexp")
                nc.scalar.activation(wuexp[:], wups[:], Exp, scale=0.125)

                K0 = lambda q: (lambda: proj_qk(1, kf[0], 0, q))
                Q0 = lambda q: (lambda: proj_qk(0, qf[0], 0, q))
                K1 = lambda q: (lambda: proj_qk(1, kf[1], 1, q))
                Q1 = lambda q: (lambda: proj_qk(0, qf[1], 1, q))
                V_ = lambda q, t: (lambda: proj_v(q, t))

                HOOKS = {
                    (0, 0): {1: [K0(1), V_(0, 0), V_(0, 1)],
                             2: [K0(2), V_(0, 2), V_(0, 3), V_(1, 0)],
                             3: [K0(3), Q0(1), V_(1, 1), V_(1, 2), V_(1, 3)]},
                    (0, 1): {0: [V_(2, 0), V_(2, 1)],
                             1: [V_(2, 2), V_(2, 3), V_(3, 0)],
                             2: [V_(3, 1), V_(3, 2), Q0(2)],
                             3: [V_(3, 3)]},
                    (0, 2): {0: [Q0(3)], 1: [K1(0)], 2: [K1(1)], 3: [K1(2)]},
                    (0, 3): {0: [K1(3)], 1: [Q1(0)], 2: [Q1(1)], 3: [Q1(2)]},
                    (1, 0): {0: [Q1(3)]},
                }

                chunks = [(0, 0), (0, 1), (0, 2), (0, 3),
                          (1, 0), (1, 1), (1, 2), (1, 3)]
                proj_qk(1, kf[0], 0, 0)
                proj_qk(0, qf[0], 0, 0)
                prev = None
                for ci, (pr, q) in enumerate(chunks):
                    last = ci == len(chunks) - 1
                    hooks = HOOKS.get((pr, q), {})
                    pts = []
                    cur = {"pr": pr, "q": q, "pts": pts}
                    for tk in range(NT):
                        g, r = divmod(tk, 4)
                        if r == 0:
                            for fn in hooks.get(g, []):
                                fn()
                            if prev is not None:
                                av_group(prev, g)
                            if last and g > 0:
                                av_group(cur, g - 1)
                        pts.append(sc_exp(pr, q, tk))
                    if prev is not None:
                        finalize(prev)
                    prev = cur
                av_group(prev, 3)
                finalize(prev)

    nc.compile()
    return nc


def _get_program():
    if "nc" not in _PROG:
        _PROG["nc"] = _build_program()
    return _PROG["nc"]


def _in_maps(x, wq_w, wq_b, wk_w, wk_b, wv_w, wv_b):
    import ml_dtypes
    bf = ml_dtypes.bfloat16
    cosf, sinf = _rope_tables()
    cosf, sinf = cosf.astype(bf), sinf.astype(bf)
    permT = _perm_matrix().astype(bf)
    ident = np.eye(128, dtype=np.float32)
    maps = []
    for c in range(NCORES):
        b, hg = divmod(c, 4)
        sl = slice(hg * CPC, (hg + 1) * CPC)
        maps.append({
            "xt": np.ascontiguousarray(x[b].T.astype(np.float32)),
            "wqt": np.ascontiguousarray(wq_w[sl].T.astype(np.float32)),
            "wkt": np.ascontiguousarray(wk_w[sl].T.astype(np.float32)),
            "wvt": np.ascontiguousarray(wv_w[sl].T.astype(np.float32)),
            "bq": wq_b[sl].reshape(1, CPC).astype(np.float32),
            "bk": wk_b[sl].reshape(1, CPC).astype(np.float32),
            "bv": wv_b[sl].reshape(1, CPC).astype(np.float32),
            "cosf": cosf, "sinf": sinf, "permT": permT, "ident": ident,
        })
    return maps


def _gather(results):
    out = np.empty((B, S, D), dtype=np.float32)
    for c in range(NCORES):
        b, hg = divmod(c, 4)
        out[b, :, hg * CPC:(hg + 1) * CPC] = results[c]["out"]
    return out


def kernel(x, wq_w, wq_b, wk_w, wk_b, wv_w, wv_b):
    from concourse.bass_utils import run_bass_kernel_spmd
    nc = _get_program()
    maps = _in_maps(x, wq_w, wq_b, wk_w, wk_b, wv_w, wv_b)
    res = run_bass_kernel_spmd(nc, maps, core_ids=list(range(NCORES)))
    return _gather(res.results)


def kernel_profiled(x, wq_w, wq_b, wk_w, wk_b, wv_w, wv_b):
    """Same as kernel() but requests an NTFF trace; returns (out, results)."""
    from concourse.bass_utils import run_bass_kernel_spmd
    nc = _get_program()
    maps = _in_maps(x, wq_w, wq_b, wk_w, wk_b, wv_w, wv_b)
    res = run_bass_kernel_spmd(
        nc, maps, core_ids=list(range(NCORES)), trace=True
    )
    return _gather(res.results), res


# revision 19
# speedup vs baseline: 407.2583x; 300.1325x over previous
"""Multi-head attention (B=2,S=2048,D=1024,H=16,HD=64) with RoPE on 8 TRN2 cores.

Sharding: core c handles batch b=c//4 and head-group hg=c%4 (4 heads = 256
output cols). Each (b, head) is independent -> no collectives.

Per-core on-chip layout (all matmul inputs bf16, PSUM f32):
  xT [1024,2048]    (D on partitions; host passes x[b].T)
  Q^T,K^T [256,2048] via matmul(lhsT=W^T tile, rhs=xT tile); bias folded in as
                     a K=1 matmul with a ones row; RoPE applied in-place via a
                     pair-swap permutation matmul + cos/sin elementwise (DVE)
  V [2048,4*65]     natural layout + a ones column per head (gives softmax
                     denominators for free during the AV matmul)
  scores^T tiles [128tk, 512q] per head = matmul(lhsT=K^T slice, rhs=Q^T slice)
                     two heads packed in the PE array via tile_position rows
  P^T = exp(scores^T * 0.125) on ACT (no max-subtraction needed: |s|<~6)
  out^T[65,512] accum = matmul(lhsT=[V_h|1], rhs=P^T)  (row 64 = denominators)
  PE-transpose out^T -> [128q, 65], multiply by 1/denominator (DVE), store f32.
"""

import numpy as np

B, S, D, H, HD = 2, 2048, 1024, 16, 64
NCORES = 8
CPC = 256  # output cols per core (4 heads)

_PROG = {}


def _rope_tables():
    i = np.arange(HD // 2, dtype=np.float64)
    theta = 10000.0 ** (-2.0 * i / HD)
    t = np.arange(S, dtype=np.float64)
    ang = np.outer(theta, t)  # [32, S]
    rowi = (np.arange(128) % 64) // 2
    cosf = np.cos(ang)[rowi].astype(np.float32)  # [128, S]
    sinf = np.sin(ang)[rowi].astype(np.float32)
    return cosf, sinf


def _perm_matrix():
    # permT[p, m]: rot[m] = sum_p permT[p, m] * q[p]
    # rot[2i] = -q[2i+1], rot[2i+1] = +q[2i]
    p = np.zeros((128, 128), dtype=np.float32)
    for i in range(64):
        p[2 * i + 1, 2 * i] = -1.0
        p[2 * i, 2 * i + 1] = 1.0
    return p


def _build_program():
    import concourse.bacc as bacc
    import concourse.mybir as mybir
    from concourse import tile

    f32 = mybir.dt.float32
    bf16 = mybir.dt.bfloat16
    mult = mybir.AluOpType.mult
    Exp = mybir.ActivationFunctionType.Exp

    nc = bacc.Bacc(None)

    xt_d = nc.declare_dram_parameter("xt", [D, S], f32, isOutput=False)
    wq_d = nc.declare_dram_parameter("wqt", [D, CPC], f32, isOutput=False)
    wk_d = nc.declare_dram_parameter("wkt", [D, CPC], f32, isOutput=False)
    wv_d = nc.declare_dram_parameter("wvt", [D, CPC], f32, isOutput=False)
    bq_d = nc.declare_dram_parameter("bq", [1, CPC], f32, isOutput=False)
    bk_d = nc.declare_dram_parameter("bk", [1, CPC], f32, isOutput=False)
    bv_d = nc.declare_dram_parameter("bv", [1, CPC], f32, isOutput=False)
    cos_d = nc.declare_dram_parameter("cosf", [128, S], bf16, isOutput=False)
    sin_d = nc.declare_dram_parameter("sinf", [128, S], bf16, isOutput=False)
    perm_d = nc.declare_dram_parameter("permT", [128, 128], bf16, isOutput=False)
    id_d = nc.declare_dram_parameter("ident", [128, 128], f32, isOutput=False)
    out_d = nc.declare_dram_parameter("out", [S, CPC], f32, isOutput=True)

    NT = S // 128   # 16 token tiles
    NQC = S // 512  # 4 query chunks
    NKT = D // 128  # 8 contraction tiles

    with tile.TileContext(nc) as tc:
        with tc.tile_pool(name="persist", bufs=1) as pp:
            # persistent SBUF tensors
            xt_sb = [pp.tile([128, NKT, 512], bf16, tag=f"xt{q}", name=f"xt{q}") for q in range(NQC)]
            w_sb = [pp.tile([128, NKT, CPC], bf16, tag=f"w{j}", name=f"w{j}") for j in range(3)]
            b_sb = [pp.tile([1, CPC], bf16, tag=f"b{j}", name=f"b{j}") for j in range(3)]
            cos_sb = pp.tile([128, S], bf16, tag="cos", name="cos")
            sin_sb = pp.tile([128, S], bf16, tag="sin", name="sin")
            perm_sb = pp.tile([128, 128], bf16, tag="perm", name="perm")
            id_sb = pp.tile([128, 128], f32, tag="ident", name="ident")
            ones_sb = pp.tile([1, 512], bf16, tag="ones", name="ones")
            qf = [pp.tile([128, S], bf16, tag=f"qf{c}", name=f"qf{c}") for c in range(2)]
            kf = [pp.tile([128, S], bf16, tag=f"kf{c}", name=f"kf{c}") for c in range(2)]
            v_sb = [pp.tile([128, 4, HD + 1], bf16, tag=f"v{t}", name=f"v{t}") for t in range(NT)]

            # ---- loads (SWDGE casts f32 -> bf16 in flight) ----
            xt_r = xt_d.rearrange("(a p) t -> p a t", p=128)
            wds = (wq_d, wk_d, wv_d)
            wk_r = wds[1].rearrange("(a p) c -> p a c", p=128)
            nc.gpsimd.dma_start(w_sb[1][:, 0:4, :], wk_r[:, 0:4, :])
            nc.gpsimd.dma_start(
                xt_sb[0][:, 0:4, :], xt_r[:, 0:4, 0:512])
            nc.gpsimd.dma_start(w_sb[1][:, 4:8, :], wk_r[:, 4:8, :])
            nc.gpsimd.dma_start(
                xt_sb[0][:, 4:8, :], xt_r[:, 4:8, 0:512])
            nc.gpsimd.dma_start(w_sb[0][:], wds[0].rearrange("(a p) c -> p a c", p=128))
            for j, bd in enumerate((bq_d, bk_d, bv_d)):
                nc.gpsimd.dma_start(b_sb[j][:], bd[:])
            for h in range(2):
                nc.gpsimd.dma_start(
                    xt_sb[1][:, h * 4:(h + 1) * 4, :],
                    xt_r[:, h * 4:(h + 1) * 4, 512:1024],
                )
            nc.gpsimd.dma_start(w_sb[2][:], wds[2].rearrange("(a p) c -> p a c", p=128))
            for q in range(2, NQC):
                for h in range(2):
                    nc.gpsimd.dma_start(
                        xt_sb[q][:, h * 4:(h + 1) * 4, :],
                        xt_r[:, h * 4:(h + 1) * 4, q * 512:(q + 1) * 512],
                    )
            nc.sync.dma_start(perm_sb[:], perm_d[:])
            nc.sync.dma_start(cos_sb[:], cos_d[:])
            nc.sync.dma_start(sin_sb[:], sin_d[:])
            nc.sync.dma_start(id_sb[:], id_d[:])
            nc.vector.memset(ones_sb[:], 1.0)


            # ---- compute: fully pipelined ----
            # K-proj(pair0) chunks feed score tiles immediately (subtile deps);
            # Q/V/pair-1 projections interleave into the attention stream.
            with (
                tc.tile_pool(name="psS", bufs=2, space="PSUM") as psS,
                tc.tile_pool(name="psX", bufs=4, space="PSUM") as psX,
                tc.tile_pool(name="stA", bufs=3) as stA,
                tc.tile_pool(name="ptp", bufs=28) as ptp,
                tc.tile_pool(name="avs", bufs=3) as avsp,
                tc.tile_pool(name="rsp", bufs=4) as rsp,
                tc.tile_pool(name="outp", bufs=1) as outp,
            ):
                def proj_qk(widx, dst, ct, q):
                    qsl = slice(q * 512, (q + 1) * 512)
                    csl = slice(ct * 128, (ct + 1) * 128)
                    ps = psX.tile([128, 512], f32, tag="av", name="ps")
                    for kt in range(NKT):
                        nc.tensor.matmul(
                            ps[:], w_sb[widx][:, kt, csl], xt_sb[q][:, kt, :],
                            start=(kt == 0), stop=False,
                        )
                    nc.tensor.matmul(
                        ps[:], b_sb[widx][0:1, csl], ones_sb[:],
                        start=False, stop=True,
                    )
                    qraw = stA.tile([128, 512], bf16, tag="qraw", name="qraw")
                    nc.vector.tensor_copy(qraw[:], ps[:])
                    nc.tensor.matmul(ps[:], perm_sb[:], qraw[:], start=True, stop=True)
                    rsb = stA.tile([128, 512], bf16, tag="rsb", name="rsb")
                    nc.vector.tensor_copy(rsb[:], ps[:])
                    t1 = stA.tile([128, 512], bf16, tag="t1", name="t1")
                    nc.vector.tensor_mul(t1[:], qraw[:], cos_sb[:, qsl])
                    t2 = stA.tile([128, 512], bf16, tag="t2", name="t2")
                    nc.vector.tensor_mul(t2[:], rsb[:], sin_sb[:, qsl])
                    nc.vector.tensor_add(dst[:, qsl], t1[:], t2[:])

                def proj_v(q, ti):
                    tt = q * 4 + ti
                    ps = psX.tile([128, CPC], f32, tag="av", name="vps")
                    for kt in range(NKT):
                        nc.tensor.matmul(
                            ps[:], xt_sb[q][:, kt, ti * 128:(ti + 1) * 128],
                            w_sb[2][:, kt, :],
                            start=(kt == 0), stop=False,
                        )
                    nc.tensor.matmul(
                        ps[:], ones_sb[0:1, 0:128], b_sb[2][:],
                        start=False, stop=True,
                    )
                    nc.vector.tensor_copy(
                        v_sb[tt][:, :, 0:HD],
                        ps[:].rearrange("p (h d) -> p h d", h=4),
                    )
                    nc.vector.memset(v_sb[tt][:, :, HD:HD + 1], 1.0)

                out_tiles = {}

                def sc_exp(pr, q, tk):
                    qsl = slice(q * 512, (q + 1) * 512)
                    tsl = slice(tk * 128, (tk + 1) * 128)
                    sc = psS.tile([128, 1024], f32, tag="sc", name="sc")
                    nc.tensor.matmul(
                        sc[:, 0:512], kf[pr][0:64, tsl], qf[pr][0:64, qsl],
                        start=True, stop=True, tile_position=(0, 0),
                    )
                    nc.tensor.matmul(
                        sc[:, 512:1024], kf[pr][64:128, tsl],
                        qf[pr][64:128, qsl],
                        start=True, stop=True, tile_position=(64, 0),
                    )
                    pt = ptp.tile([128, 1024], bf16, tag="pt", name="pt")
                    nc.scalar.activation(pt[:], sc[:], Exp, scale=0.125)
                    return pt

                def av_group(st, g):
                    if g == 0:
                        st["av_e"] = psX.tile([HD + 1, 512], f32, tag="av", name="av_e")
                        st["av_o"] = psX.tile([HD + 1, 512], f32, tag="av", name="av_o")
                    pr = st["pr"]
                    for tk in range(4 * g, 4 * g + 4):
                        nc.tensor.matmul(
                            st["av_e"][:], v_sb[tk][:, 2 * pr, :],
                            st["pts"][tk][:, 0:512],
                            start=(tk == 0), stop=(tk == NT - 1),
                            skip_group_check=True,
                        )
                        nc.tensor.matmul(
                            st["av_o"][:], v_sb[tk][:, 2 * pr + 1, :],
                            st["pts"][tk][:, 512:1024],
                            start=(tk == 0), stop=(tk == NT - 1),
                            skip_group_check=True,
                        )

                def finalize(st):
                    pr, q = st["pr"], st["q"]
                    for e, av in ((0, st["av_e"]), (1, st["av_o"])):
                        hc = 2 * pr + e
                        avs = avsp.tile([HD + 1, 512], f32, tag="avs", name="avs")
                        nc.any.tensor_copy(avs[:], av[:])
                        for j in range(4):
                            qt = q * 4 + j
                            if qt not in out_tiles:
                                out_tiles[qt] = outp.tile(
                                    [128, CPC], f32, tag=f"o{qt}", name=f"o{qt}"
                                )
                            tp = psX.tile([128, HD + 1], f32, tag="av", name="tp")
                            nc.tensor.transpose(
                                tp[:], avs[:, j * 128:(j + 1) * 128],
                                id_sb[0:HD + 1, 0:HD + 1],
                            )
                            rs = rsp.tile([128, 1], f32, tag="rs", name="rs")
                            nc.vector.reciprocal_approx_fast(rs[:], tp[:, HD:HD + 1])
                            nc.vector.tensor_scalar_mul(
                                out_tiles[qt][:, hc * HD:(hc + 1) * HD],
                                tp[:, 0:HD], rs[:],
                            )
                    if pr == 1:
                        for j in range(4):
                            qt = q * 4 + j
                            r0 = qt * 128
                            nc.sync.dma_start(out_d[r0:r0 + 128, :],
                                              out_tiles[qt][:])

                # warmup: PE busy + ACT exp table preload while DMAs land
                wu = stA.tile([128, 512], bf16, tag="qraw", name="wu")
                nc.vector.memset(wu[:], 0.0)
                wups = psX.tile([128, 512], f32, tag="av", name="wups")
                for _ in range(16):
                    nc.tensor.matmul(wups[:], wu[:, 0:128], wu[:],
                                     start=True, stop=True)
                wuexp = stA.tile([128, 512], bf16, tag="rsb", name="wuexp")
                nc.scalar.activation(wuexp[:], wups[:], Exp, scale=0.125)

                K0 = lambda q: (lambda: proj_qk(1, kf[0], 0, q))
                Q0 = lambda q: (lambda: proj_qk(0, qf[0], 0, q))
                K1 = lambda q: (lambda: proj_qk(1, kf[1], 1, q))
                Q1 = lambda q: (lambda: proj_qk(0, qf[1], 1, q))
                V_ = lambda q, t: (lambda: proj_v(q, t))

                HOOKS = {
                    (0, 0): {1: [K0(1), V_(0, 0), V_(0, 1)],
                             2: [K0(2), V_(0, 2), V_(0, 3), V_(1, 0)],
                             3: [K0(3), Q0(1)]},
                    (0, 1): {0: [V_(1, 1), V_(1, 2), V_(1, 3)],
                             1: [V_(2, 0), V_(2, 1), V_(2, 2), V_(2, 3)],
                             2: [V_(3, 0), V_(3, 1), Q0(2)],
                             3: [V_(3, 2), V_(3, 3)]},
                    (0, 2): {0: [Q0(3)], 1: [K1(0)], 2: [K1(1)], 3: [K1(2)]},
                    (0, 3): {0: [K1(3)], 1: [Q1(0)], 2: [Q1(1)], 3: [Q1(2)]},
                    (1, 0): {0: [Q1(3)]},
                }

                chunks = [(0, 0), (0, 1), (0, 2), (0, 3),
                          (1, 0), (1, 1), (1, 2), (1, 3)]
                proj_qk(1, kf[0], 0, 0)
                proj_qk(0, qf[0], 0, 0)
                prev = None
                for ci, (pr, q) in enumerate(chunks):
                    last = ci == len(chunks) - 1
                    hooks = HOOKS.get((pr, q), {})
                    pts = []
                    cur = {"pr": pr, "q": q, "pts": pts}
                    for tk in range(NT):
                        g, r = divmod(tk, 4)
                        if r == 0:
                            for fn in hooks.get(g, []):
                                fn()
                            if prev is not None:
                                av_group(prev, g)
                            if last and g > 0:
                                av_group(cur, g - 1)
                        pts.append(sc_exp(pr, q, tk))
                    if prev is not None:
                        finalize(prev)
                    prev = cur
                av_group(prev, 3)
                finalize(prev)

    nc.compile()
    return nc


def _get_program():
    if "nc" not in _PROG:
        _PROG["nc"] = _build_program()
    return _PROG["nc"]


def _in_maps(x, wq_w, wq_b, wk_w, wk_b, wv_w, wv_b):
    import ml_dtypes
    bf = ml_dtypes.bfloat16
    cosf, sinf = _rope_tables()
    cosf, sinf = cosf.astype(bf), sinf.astype(bf)
    permT = _perm_matrix().astype(bf)
    ident = np.eye(128, dtype=np.float32)
    maps = []
    for c in range(NCORES):
        b, hg = divmod(c, 4)
        sl = slice(hg * CPC, (hg + 1) * CPC)
        maps.append({
            "xt": np.ascontiguousarray(x[b].T.astype(np.float32)),
            "wqt": np.ascontiguousarray(wq_w[sl].T.astype(np.float32)),
            "wkt": np.ascontiguousarray(wk_w[sl].T.astype(np.float32)),
            "wvt": np.ascontiguousarray(wv_w[sl].T.astype(np.float32)),
            "bq": wq_b[sl].reshape(1, CPC).astype(np.float32),
            "bk": wk_b[sl].reshape(1, CPC).astype(np.float32),
            "bv": wv_b[sl].reshape(1, CPC).astype(np.float32),
            "cosf": cosf, "sinf": sinf, "permT": permT, "ident": ident,
        })
    return maps


def _gather(results):
    out = np.empty((B, S, D), dtype=np.float32)
    for c in range(NCORES):
        b, hg = divmod(c, 4)
        out[b, :, hg * CPC:(hg + 1) * CPC] = results[c]["out"]
    return out


def kernel(x, wq_w, wq_b, wk_w, wk_b, wv_w, wv_b):
    from concourse.bass_utils import run_bass_kernel_spmd
    x = np.asarray(x, np.float32)
    wq_w, wq_b = np.asarray(wq_w, np.float32), np.asarray(wq_b, np.float32)
    wk_w, wk_b = np.asarray(wk_w, np.float32), np.asarray(wk_b, np.float32)
    wv_w, wv_b = np.asarray(wv_w, np.float32), np.asarray(wv_b, np.float32)
    nc = _get_program()
    maps = _in_maps(x, wq_w, wq_b, wk_w, wk_b, wv_w, wv_b)
    res = run_bass_kernel_spmd(nc, maps, core_ids=list(range(NCORES)))
    return _gather(res.results)


def kernel_profiled(x, wq_w, wq_b, wk_w, wk_b, wv_w, wv_b):
    """Same as kernel() but requests an NTFF trace; returns (out, results)."""
    from concourse.bass_utils import run_bass_kernel_spmd
    nc = _get_program()
    maps = _in_maps(x, wq_w, wq_b, wk_w, wk_b, wv_w, wv_b)
    res = run_bass_kernel_spmd(
        nc, maps, core_ids=list(range(NCORES)), trace=True
    )
    return _gather(res.results), res


# revision 20
# speedup vs baseline: 407.6734x; 1.0010x over previous
"""Multi-head attention (B=2,S=2048,D=1024,H=16,HD=64) with RoPE on 8 TRN2 cores.

Sharding: core c handles batch b=c//4 and head-group hg=c%4 (4 heads = 256
output cols). Each (b, head) is independent -> no collectives.

Per-core on-chip layout (all matmul inputs bf16, PSUM f32):
  xT [1024,2048]    (D on partitions; host passes x[b].T)
  Q^T,K^T [256,2048] via matmul(lhsT=W^T tile, rhs=xT tile); bias folded in as
                     a K=1 matmul with a ones row; RoPE applied in-place via a
                     pair-swap permutation matmul + cos/sin elementwise (DVE)
  V [2048,4*65]     natural layout + a ones column per head (gives softmax
                     denominators for free during the AV matmul)
  scores^T tiles [128tk, 512q] per head = matmul(lhsT=K^T slice, rhs=Q^T slice)
                     two heads packed in the PE array via tile_position rows
  P^T = exp(scores^T * 0.125) on ACT (no max-subtraction needed: |s|<~6)
  out^T[65,512] accum = matmul(lhsT=[V_h|1], rhs=P^T)  (row 64 = denominators)
  PE-transpose out^T -> [128q, 65], multiply by 1/denominator (DVE), store f32.
"""

import numpy as np

B, S, D, H, HD = 2, 2048, 1024, 16, 64
NCORES = 8
CPC = 256  # output cols per core (4 heads)

_PROG = {}


def _rope_tables():
    i = np.arange(HD // 2, dtype=np.float64)
    theta = 10000.0 ** (-2.0 * i / HD)
    t = np.arange(S, dtype=np.float64)
    ang = np.outer(theta, t)  # [32, S]
    rowi = (np.arange(128) % 64) // 2
    cosf = np.cos(ang)[rowi].astype(np.float32)  # [128, S]
    sinf = np.sin(ang)[rowi].astype(np.float32)
    return cosf, sinf


def _perm_matrix():
    # permT[p, m]: rot[m] = sum_p permT[p, m] * q[p]
    # rot[2i] = -q[2i+1], rot[2i+1] = +q[2i]
    p = np.zeros((128, 128), dtype=np.float32)
    for i in range(64):
        p[2 * i + 1, 2 * i] = -1.0
        p[2 * i, 2 * i + 1] = 1.0
    return p


def _build_program():
    import concourse.bacc as bacc
    import concourse.mybir as mybir
    from concourse import tile

    f32 = mybir.dt.float32
    bf16 = mybir.dt.bfloat16
    mult = mybir.AluOpType.mult
    Exp = mybir.ActivationFunctionType.Exp

    nc = bacc.Bacc(None)

    xt_d = nc.declare_dram_parameter("xt", [D, S], f32, isOutput=False)
    wq_d = nc.declare_dram_parameter("wqt", [D, CPC], f32, isOutput=False)
    wk_d = nc.declare_dram_parameter("wkt", [D, CPC], f32, isOutput=False)
    wv_d = nc.declare_dram_parameter("wvt", [D, CPC], f32, isOutput=False)
    bq_d = nc.declare_dram_parameter("bq", [1, CPC], f32, isOutput=False)
    bk_d = nc.declare_dram_parameter("bk", [1, CPC], f32, isOutput=False)
    bv_d = nc.declare_dram_parameter("bv", [1, CPC], f32, isOutput=False)
    cos_d = nc.declare_dram_parameter("cosf", [128, S], bf16, isOutput=False)
    sin_d = nc.declare_dram_parameter("sinf", [128, S], bf16, isOutput=False)
    perm_d = nc.declare_dram_parameter("permT", [128, 128], bf16, isOutput=False)
    id_d = nc.declare_dram_parameter("ident", [128, 128], f32, isOutput=False)
    out_d = nc.declare_dram_parameter("out", [S, CPC], f32, isOutput=True)

    NT = S // 128   # 16 token tiles
    NQC = S // 512  # 4 query chunks
    NKT = D // 128  # 8 contraction tiles

    with tile.TileContext(nc) as tc:
        with tc.tile_pool(name="persist", bufs=1) as pp:
            # persistent SBUF tensors
            xt_sb = [pp.tile([128, NKT, 512], bf16, tag=f"xt{q}", name=f"xt{q}") for q in range(NQC)]
            w_sb = [pp.tile([128, NKT, CPC], bf16, tag=f"w{j}", name=f"w{j}") for j in range(3)]
            b_sb = [pp.tile([1, CPC], bf16, tag=f"b{j}", name=f"b{j}") for j in range(3)]
            cos_sb = pp.tile([128, S], bf16, tag="cos", name="cos")
            sin_sb = pp.tile([128, S], bf16, tag="sin", name="sin")
            perm_sb = pp.tile([128, 128], bf16, tag="perm", name="perm")
            id_sb = pp.tile([128, 128], f32, tag="ident", name="ident")
            ones_sb = pp.tile([1, 512], bf16, tag="ones", name="ones")
            qf = [pp.tile([128, S], bf16, tag=f"qf{c}", name=f"qf{c}") for c in range(2)]
            kf = [pp.tile([128, S], bf16, tag=f"kf{c}", name=f"kf{c}") for c in range(2)]
            v_sb = [pp.tile([128, 4, HD + 1], bf16, tag=f"v{t}", name=f"v{t}") for t in range(NT)]

            # ---- loads (SWDGE casts f32 -> bf16 in flight) ----
            xt_r = xt_d.rearrange("(a p) t -> p a t", p=128)
            wds = (wq_d, wk_d, wv_d)
            wk_r = wds[1].rearrange("(a p) c -> p a c", p=128)
            wq_r = wds[0].rearrange("(a p) c -> p a c", p=128)
            nc.gpsimd.dma_start(w_sb[1][:, 0:4, :], wk_r[:, 0:4, :])
            nc.gpsimd.dma_start(
                xt_sb[0][:, 0:4, :], xt_r[:, 0:4, 0:512])
            nc.gpsimd.dma_start(w_sb[0][:, 0:4, :], wq_r[:, 0:4, :])
            nc.gpsimd.dma_start(w_sb[1][:, 4:8, :], wk_r[:, 4:8, :])
            nc.gpsimd.dma_start(
                xt_sb[0][:, 4:8, :], xt_r[:, 4:8, 0:512])
            nc.gpsimd.dma_start(w_sb[0][:, 4:8, :], wq_r[:, 4:8, :])
            for j, bd in enumerate((bq_d, bk_d, bv_d)):
                nc.gpsimd.dma_start(b_sb[j][:], bd[:])
            for h in range(2):
                nc.gpsimd.dma_start(
                    xt_sb[1][:, h * 4:(h + 1) * 4, :],
                    xt_r[:, h * 4:(h + 1) * 4, 512:1024],
                )
            nc.gpsimd.dma_start(w_sb[2][:], wds[2].rearrange("(a p) c -> p a c", p=128))
            for q in range(2, NQC):
                for h in range(2):
                    nc.gpsimd.dma_start(
                        xt_sb[q][:, h * 4:(h + 1) * 4, :],
                        xt_r[:, h * 4:(h + 1) * 4, q * 512:(q + 1) * 512],
                    )
            nc.sync.dma_start(perm_sb[:], perm_d[:])
            nc.sync.dma_start(cos_sb[:], cos_d[:])
            nc.sync.dma_start(sin_sb[:], sin_d[:])
            nc.sync.dma_start(id_sb[:], id_d[:])
            nc.vector.memset(ones_sb[:], 1.0)


            # ---- compute: fully pipelined ----
            # K-proj(pair0) chunks feed score tiles immediately (subtile deps);
            # Q/V/pair-1 projections interleave into the attention stream.
            with (
                tc.tile_pool(name="psS", bufs=2, space="PSUM") as psS,
                tc.tile_pool(name="psX", bufs=4, space="PSUM") as psX,
                tc.tile_pool(name="stA", bufs=3) as stA,
                tc.tile_pool(name="ptp", bufs=28) as ptp,
                tc.tile_pool(name="avs", bufs=3) as avsp,
                tc.tile_pool(name="rsp", bufs=4) as rsp,
                tc.tile_pool(name="outp", bufs=1) as outp,
            ):
                def proj_qk(widx, dst, ct, q):
                    qsl = slice(q * 512, (q + 1) * 512)
                    csl = slice(ct * 128, (ct + 1) * 128)
                    ps = psX.tile([128, 512], f32, tag="av", name="ps")
                    for kt in range(NKT):
                        nc.tensor.matmul(
                            ps[:], w_sb[widx][:, kt, csl], xt_sb[q][:, kt, :],
                            start=(kt == 0), stop=False,
                        )
                    nc.tensor.matmul(
                        ps[:], b_sb[widx][0:1, csl], ones_sb[:],
                        start=False, stop=True,
                    )
                    qraw = stA.tile([128, 512], bf16, tag="qraw", name="qraw")
                    nc.vector.tensor_copy(qraw[:], ps[:])
                    nc.tensor.matmul(ps[:], perm_sb[:], qraw[:], start=True, stop=True)
                    rsb = stA.tile([128, 512], bf16, tag="rsb", name="rsb")
                    nc.vector.tensor_copy(rsb[:], ps[:])
                    t1 = stA.tile([128, 512], bf16, tag="t1", name="t1")
                    nc.vector.tensor_mul(t1[:], qraw[:], cos_sb[:, qsl])
                    t2 = stA.tile([128, 512], bf16, tag="t2", name="t2")
                    nc.vector.tensor_mul(t2[:], rsb[:], sin_sb[:, qsl])
                    nc.vector.tensor_add(dst[:, qsl], t1[:], t2[:])

                def proj_v(q, ti):
                    tt = q * 4 + ti
                    ps = psX.tile([128, CPC], f32, tag="av", name="vps")
                    for kt in range(NKT):
                        nc.tensor.matmul(
                            ps[:], xt_sb[q][:, kt, ti * 128:(ti + 1) * 128],
                            w_sb[2][:, kt, :],
                            start=(kt == 0), stop=False,
                        )
                    nc.tensor.matmul(
                        ps[:], ones_sb[0:1, 0:128], b_sb[2][:],
                        start=False, stop=True,
                    )
                    nc.vector.tensor_copy(
                        v_sb[tt][:, :, 0:HD],
                        ps[:].rearrange("p (h d) -> p h d", h=4),
                    )
                    nc.vector.memset(v_sb[tt][:, :, HD:HD + 1], 1.0)

                out_tiles = {}

                def sc_exp(pr, q, tk):
                    qsl = slice(q * 512, (q + 1) * 512)
                    tsl = slice(tk * 128, (tk + 1) * 128)
                    sc = psS.tile([128, 1024], f32, tag="sc", name="sc")
                    nc.tensor.matmul(
                        sc[:, 0:512], kf[pr][0:64, tsl], qf[pr][0:64, qsl],
                        start=True, stop=True, tile_position=(0, 0),
                    )
                    nc.tensor.matmul(
                        sc[:, 512:1024], kf[pr][64:128, tsl],
                        qf[pr][64:128, qsl],
                        start=True, stop=True, tile_position=(64, 0),
                    )
                    pt = ptp.tile([128, 1024], bf16, tag="pt", name="pt")
                    nc.scalar.activation(pt[:], sc[:], Exp, scale=0.125)
                    return pt

                def av_group(st, g):
                    if g == 0:
                        st["av_e"] = psX.tile([HD + 1, 512], f32, tag="av", name="av_e")
                        st["av_o"] = psX.tile([HD + 1, 512], f32, tag="av", name="av_o")
                    pr = st["pr"]
                    for tk in range(4 * g, 4 * g + 4):
                        nc.tensor.matmul(
                            st["av_e"][:], v_sb[tk][:, 2 * pr, :],
                            st["pts"][tk][:, 0:512],
                            start=(tk == 0), stop=(tk == NT - 1),
                            skip_group_check=True,
                        )
                        nc.tensor.matmul(
                            st["av_o"][:], v_sb[tk][:, 2 * pr + 1, :],
                            st["pts"][tk][:, 512:1024],
                            start=(tk == 0), stop=(tk == NT - 1),
                            skip_group_check=True,
                        )

                def finalize(st):
                    pr, q = st["pr"], st["q"]
                    for e, av in ((0, st["av_e"]), (1, st["av_o"])):
                        hc = 2 * pr + e
                        avs = avsp.tile([HD + 1, 512], f32, tag="avs", name="avs")
                        nc.any.tensor_copy(avs[:], av[:])
                        for j in range(4):
                            qt = q * 4 + j
                            if qt not in out_tiles:
                                out_tiles[qt] = outp.tile(
                                    [128, CPC], f32, tag=f"o{qt}", name=f"o{qt}"
                                )
                            tp = psX.tile([128, HD + 1], f32, tag="av", name="tp")
                            nc.tensor.transpose(
                                tp[:], avs[:, j * 128:(j + 1) * 128],
                                id_sb[0:HD + 1, 0:HD + 1],
                            )
                            rs = rsp.tile([128, 1], f32, tag="rs", name="rs")
                            nc.vector.reciprocal_approx_fast(rs[:], tp[:, HD:HD + 1])
                            nc.vector.tensor_scalar_mul(
                                out_tiles[qt][:, hc * HD:(hc + 1) * HD],
                                tp[:, 0:HD], rs[:],
                            )
                    if pr == 1:
                        for j in range(4):
                            qt = q * 4 + j
                            r0 = qt * 128
                            nc.sync.dma_start(out_d[r0:r0 + 128, :],
                                              out_tiles[qt][:])

                # warmup: PE busy + ACT exp table preload while DMAs land
                wu = stA.tile([128, 512], bf16, tag="qraw", name="wu")
                nc.vector.memset(wu[:], 0.0)
                wups = psX.tile([128, 512], f32, tag="av", name="wups")
                for _ in range(16):
                    nc.tensor.matmul(wups[:], wu[:, 0:128], wu[:],
                                     start=True, stop=True)
                wuexp = stA.tile([128, 512], bf16, tag="rsb", name="wuexp")
                nc.scalar.activation(wuexp[:], wups[:], Exp, scale=0.125)

                K0 = lambda q: (lambda: proj_qk(1, kf[0], 0, q))
                Q0 = lambda q: (lambda: proj_qk(0, qf[0], 0, q))
                K1 = lambda q: (lambda: proj_qk(1, kf[1], 1, q))
                Q1 = lambda q: (lambda: proj_qk(0, qf[1], 1, q))
                V_ = lambda q, t: (lambda: proj_v(q, t))

                HOOKS = {
                    (0, 0): {1: [K0(1), V_(0, 0), V_(0, 1)],
                             2: [K0(2), V_(0, 2), V_(0, 3), V_(1, 0)],
                             3: [K0(3), Q0(1)]},
                    (0, 1): {0: [V_(1, 1), V_(1, 2), V_(1, 3)],
                             1: [V_(2, 0), V_(2, 1), V_(2, 2), V_(2, 3)],
                             2: [V_(3, 0), V_(3, 1), Q0(2)],
                             3: [V_(3, 2), V_(3, 3)]},
                    (0, 2): {0: [Q0(3)], 1: [K1(0)], 2: [K1(1)], 3: [K1(2)]},
                    (0, 3): {0: [K1(3)], 1: [Q1(0)], 2: [Q1(1)], 3: [Q1(2)]},
                    (1, 0): {0: [Q1(3)]},
                }

                chunks = [(0, 0), (0, 1), (0, 2), (0, 3),
                          (1, 0), (1, 1), (1, 2), (1, 3)]
                proj_qk(1, kf[0], 0, 0)
                proj_qk(0, qf[0], 0, 0)
                prev = None
                for ci, (pr, q) in enumerate(chunks):
                    last = ci == len(chunks) - 1
                    hooks = HOOKS.get((pr, q), {})
                    pts = []
                    cur = {"pr": pr, "q": q, "pts": pts}
                    for tk in range(NT):
                        g, r = divmod(tk, 4)
                        if r == 0:
                            for fn in hooks.get(g, []):
                                fn()
                            if prev is not None:
                                av_group(prev, g)
                            if last and g > 0:
                                av_group(cur, g - 1)
                        pts.append(sc_exp(pr, q, tk))
                    if prev is not None:
                        finalize(prev)
                    prev = cur
                av_group(prev, 3)
                finalize(prev)

    nc.compile()
    return nc


def _get_program():
    if "nc" not in _PROG:
        _PROG["nc"] = _build_program()
    return _PROG["nc"]


def _in_maps(x, wq_w, wq_b, wk_w, wk_b, wv_w, wv_b):
    import ml_dtypes
    bf = ml_dtypes.bfloat16
    cosf, sinf = _rope_tables()
    cosf, sinf = cosf.astype(bf), sinf.astype(bf)
    permT = _perm_matrix().astype(bf)
    ident = np.eye(128, dtype=np.float32)
    maps = []
    for c in range(NCORES):
        b, hg = divmod(c, 4)
        sl = slice(hg * CPC, (hg + 1) * CPC)
        maps.append({
            "xt": np.ascontiguousarray(x[b].T.astype(np.float32)),
            "wqt": np.ascontiguousarray(wq_w[sl].T.astype(np.float32)),
            "wkt": np.ascontiguousarray(wk_w[sl].T.astype(np.float32)),
            "wvt": np.ascontiguousarray(wv_w[sl].T.astype(np.float32)),
            "bq": wq_b[sl].reshape(1, CPC).astype(np.float32),
            "bk": wk_b[sl].reshape(1, CPC).astype(np.float32),
            "bv": wv_b[sl].reshape(1, CPC).astype(np.float32),
            "cosf": cosf, "sinf": sinf, "permT": permT, "ident": ident,
        })
    return maps


def _gather(results):
    out = np.empty((B, S, D), dtype=np.float32)
    for c in range(NCORES):
        b, hg = divmod(c, 4)
        out[b, :, hg * CPC:(hg + 1) * CPC] = results[c]["out"]
    return out


def kernel(x, wq_w, wq_b, wk_w, wk_b, wv_w, wv_b):
    from concourse.bass_utils import run_bass_kernel_spmd
    x = np.asarray(x, np.float32)
    wq_w, wq_b = np.asarray(wq_w, np.float32), np.asarray(wq_b, np.float32)
    wk_w, wk_b = np.asarray(wk_w, np.float32), np.asarray(wk_b, np.float32)
    wv_w, wv_b = np.asarray(wv_w, np.float32), np.asarray(wv_b, np.float32)
    nc = _get_program()
    maps = _in_maps(x, wq_w, wq_b, wk_w, wk_b, wv_w, wv_b)
    res = run_bass_kernel_spmd(nc, maps, core_ids=list(range(NCORES)))
    return _gather(res.results)


def kernel_profiled(x, wq_w, wq_b, wk_w, wk_b, wv_w, wv_b):
    """Same as kernel() but requests an NTFF trace; returns (out, results)."""
    from concourse.bass_utils import run_bass_kernel_spmd
    nc = _get_program()
    maps = _in_maps(x, wq_w, wq_b, wk_w, wk_b, wv_w, wv_b)
    res = run_bass_kernel_spmd(
        nc, maps, core_ids=list(range(NCORES)), trace=True
    )
    return _gather(res.results), res


# revision 38
# speedup vs baseline: 428.9402x; 1.0522x over previous
"""Multi-head attention (B=2,S=2048,D=1024,H=16,HD=64) with RoPE on 8 TRN2 cores.

Sharding: core c handles batch b=c//4 and head-group hg=c%4 (4 heads = 256
output cols). Each (b, head) is independent -> no collectives.

Per-core on-chip layout (all matmul inputs bf16, PSUM f32):
  xT [1024,2048]    (D on partitions; host passes x[b].T)
  Q^T,K^T [256,2048] via matmul(lhsT=W^T tile, rhs=xT tile); bias fused into
                     the DVE PSUM->SBUF copy (per-partition tensor_scalar add);
                     RoPE in-place via a pair-swap permutation matmul +
                     cos/sin elementwise (DVE, mixed f32-PSUM x bf16 inputs)
  V [2048,4*65]     natural layout, bias fused in the copy, plus a memset ones
                     column per head (softmax denominators fall out of the AV
                     matmul as row 64)
  scores^T tiles [128tk, 512q] per head = matmul(lhsT=K^T slice, rhs=Q^T slice)
                     two heads packed in the PE array via tile_position rows
  P^T = exp(scores^T * 0.125) on ACT (no max-subtraction needed: |s|<~6)
  out^T[65,512] accum = matmul(lhsT=[V_h|1], rhs=P^T)  (row 64 = denominators)
  PE-transpose out^T -> [128q, 65], multiply by 1/denominator (DVE), store f32.
"""

import numpy as np

B, S, D, H, HD = 2, 2048, 1024, 16, 64
NCORES = 8
CPC = 256  # output cols per core (4 heads)

_PROG = {}


def _rope_tables():
    i = np.arange(HD // 2, dtype=np.float64)
    theta = 10000.0 ** (-2.0 * i / HD)
    t = np.arange(S, dtype=np.float64)
    ang = np.outer(theta, t)  # [32, S]
    rowi = (np.arange(128) % 64) // 2
    cosf = np.cos(ang)[rowi].astype(np.float32)  # [128, S]
    sinf = np.sin(ang)[rowi].astype(np.float32)
    return cosf, sinf


def _perm_matrix():
    # permT[p, m]: rot[m] = sum_p permT[p, m] * q[p]
    # rot[2i] = -q[2i+1], rot[2i+1] = +q[2i]
    p = np.zeros((128, 128), dtype=np.float32)
    for i in range(64):
        p[2 * i + 1, 2 * i] = -1.0
        p[2 * i, 2 * i + 1] = 1.0
    return p


def _build_program():
    import concourse.bacc as bacc
    import concourse.mybir as mybir
    from concourse import tile

    f32 = mybir.dt.float32
    bf16 = mybir.dt.bfloat16
    mult = mybir.AluOpType.mult
    Exp = mybir.ActivationFunctionType.Exp

    nc = bacc.Bacc(None)

    xt_d = nc.declare_dram_parameter("xt", [D, S], f32, isOutput=False)
    wq_d = nc.declare_dram_parameter("wqt", [D, CPC], f32, isOutput=False)
    wk_d = nc.declare_dram_parameter("wkt", [D, CPC], f32, isOutput=False)
    wv_d = nc.declare_dram_parameter("wvt", [D, CPC], f32, isOutput=False)
    bp_d = nc.declare_dram_parameter("biasp", [128, 4], f32, isOutput=False)
    bv_d = nc.declare_dram_parameter("biasv", [128, CPC], bf16, isOutput=False)
    cos_d = nc.declare_dram_parameter("cosf", [128, S], bf16, isOutput=False)
    sin_d = nc.declare_dram_parameter("sinf", [128, S], bf16, isOutput=False)
    perm_d = nc.declare_dram_parameter("permT", [128, 128], bf16, isOutput=False)
    id_d = nc.declare_dram_parameter("ident", [128, 128], f32, isOutput=False)
    out_d = nc.declare_dram_parameter("out", [S, CPC], f32, isOutput=True)

    NT = S // 128   # 16 token tiles
    NQC = S // 512  # 4 query chunks
    NKT = D // 128  # 8 contraction tiles

    with tile.TileContext(nc) as tc:
        with tc.tile_pool(name="persist", bufs=1) as pp:
            # persistent SBUF tensors
            xt_sb = [pp.tile([128, NKT, 512], bf16, tag=f"xt{q}", name=f"xt{q}") for q in range(NQC)]
            w_sb = [pp.tile([128, NKT, CPC], bf16, tag=f"w{j}", name=f"w{j}") for j in range(3)]
            bp_sb = pp.tile([128, 4], f32, tag="biasp", name="biasp")
            bv_sb = pp.tile([128, CPC], bf16, tag="biasv", name="biasv")
            cos_sb = pp.tile([128, S], bf16, tag="cos", name="cos")
            sin_sb = pp.tile([128, S], bf16, tag="sin", name="sin")
            perm_sb = pp.tile([128, 128], bf16, tag="perm", name="perm")
            id_sb = pp.tile([128, 128], f32, tag="ident", name="ident")
            qf = [pp.tile([128, S], bf16, tag=f"qf{c}", name=f"qf{c}") for c in range(2)]
            kf = [pp.tile([128, S], bf16, tag=f"kf{c}", name=f"kf{c}") for c in range(2)]
            v_sb = [pp.tile([128, 4, HD + 1], bf16, tag=f"v{t}", name=f"v{t}") for t in range(NT)]

            # ---- loads (SWDGE casts f32 -> bf16 in flight) ----
            xt_r = xt_d.rearrange("(a p) t -> p a t", p=128)
            wds = (wq_d, wk_d, wv_d)
            wk_r = wds[1].rearrange("(a p) c -> p a c", p=128)
            wq_r = wds[0].rearrange("(a p) c -> p a c", p=128)
            nc.gpsimd.dma_start(w_sb[1][:, 0:4, :], wk_r[:, 0:4, :])
            nc.gpsimd.dma_start(
                xt_sb[0][:, 0:4, :], xt_r[:, 0:4, 0:512])
            nc.gpsimd.dma_start(w_sb[0][:, 0:4, :], wq_r[:, 0:4, :])
            nc.gpsimd.dma_start(w_sb[1][:, 4:8, :], wk_r[:, 4:8, :])
            nc.gpsimd.dma_start(
                xt_sb[0][:, 4:8, :], xt_r[:, 4:8, 0:512])
            nc.gpsimd.dma_start(w_sb[0][:, 4:8, :], wq_r[:, 4:8, :])
            for h in range(2):
                nc.gpsimd.dma_start(
                    xt_sb[1][:, h * 4:(h + 1) * 4, :],
                    xt_r[:, h * 4:(h + 1) * 4, 512:1024],
                )
            nc.gpsimd.dma_start(w_sb[2][:], wds[2].rearrange("(a p) c -> p a c", p=128))
            for q in range(2, NQC):
                for h in range(2):
                    nc.gpsimd.dma_start(
                        xt_sb[q][:, h * 4:(h + 1) * 4, :],
                        xt_r[:, h * 4:(h + 1) * 4, q * 512:(q + 1) * 512],
                    )
            nc.sync.dma_start(bp_sb[:], bp_d[:])
            nc.sync.dma_start(bv_sb[:], bv_d[:])
            nc.sync.dma_start(perm_sb[:], perm_d[:])
            nc.sync.dma_start(cos_sb[:], cos_d[:])
            nc.sync.dma_start(sin_sb[:], sin_d[:])
            nc.sync.dma_start(id_sb[:], id_d[:])


            # ---- compute: fully pipelined ----
            # K-proj(pair0) chunks feed score tiles immediately (subtile deps);
            # Q/V/pair-1 projections interleave into the attention stream.
            with (
                tc.tile_pool(name="psS", bufs=2, space="PSUM") as psS,
                tc.tile_pool(name="psX", bufs=4, space="PSUM") as psX,
                tc.tile_pool(name="stA", bufs=3) as stA,
                tc.tile_pool(name="ptp", bufs=28) as ptp,
                tc.tile_pool(name="avs", bufs=3) as avsp,
                tc.tile_pool(name="rsp", bufs=4) as rsp,
                tc.tile_pool(name="outp", bufs=1) as outp,
            ):
                def proj_qk(widx, dst, ct, q):
                    qsl = slice(q * 512, (q + 1) * 512)
                    csl = slice(ct * 128, (ct + 1) * 128)
                    ps = psX.tile([128, 512], f32, tag="av", name="ps")
                    for kt in range(NKT):
                        nc.tensor.matmul(
                            ps[:], w_sb[widx][:, kt, csl], xt_sb[q][:, kt, :],
                            start=(kt == 0), stop=(kt == NKT - 1),
                        )
                    qraw = stA.tile([128, 512], bf16, tag="qraw", name="qraw")
                    nc.vector.tensor_scalar(
                        qraw[:], ps[:], bp_sb[:, widx * 2 + ct:widx * 2 + ct + 1],
                        None, mybir.AluOpType.add,
                    )
                    nc.tensor.matmul(ps[:], perm_sb[:], qraw[:], start=True, stop=True)
                    t1 = stA.tile([128, 512], bf16, tag="t1", name="t1")
                    nc.vector.tensor_mul(t1[:], qraw[:], cos_sb[:, qsl])
                    t2 = stA.tile([128, 512], bf16, tag="t2", name="t2")
                    nc.vector.tensor_mul(t2[:], ps[:], sin_sb[:, qsl])
                    nc.vector.tensor_add(dst[:, qsl], t1[:], t2[:])

                def proj_v(q, ti):
                    tt = q * 4 + ti
                    ps = psX.tile([128, CPC], f32, tag="av", name="vps")
                    for kt in range(NKT):
                        nc.tensor.matmul(
                            ps[:], xt_sb[q][:, kt, ti * 128:(ti + 1) * 128],
                            w_sb[2][:, kt, :],
                            start=(kt == 0), stop=(kt == NKT - 1),
                        )
                    nc.vector.tensor_add(
                        v_sb[tt][:, :, 0:HD],
                        ps[:].rearrange("p (h d) -> p h d", h=4),
                        bv_sb[:].rearrange("p (h d) -> p h d", h=4),
                    )
                    nc.vector.memset(v_sb[tt][:, :, HD:HD + 1], 1.0)

                out_tiles = {}

                def sc_exp(pr, q, tk):
                    qsl = slice(q * 512, (q + 1) * 512)
                    tsl = slice(tk * 128, (tk + 1) * 128)
                    sc = psS.tile([128, 1024], f32, tag="sc", name="sc")
                    nc.tensor.matmul(
                        sc[:, 0:512], kf[pr][0:64, tsl], qf[pr][0:64, qsl],
                        start=True, stop=True, tile_position=(0, 0),
                    )
                    nc.tensor.matmul(
                        sc[:, 512:1024], kf[pr][64:128, tsl],
                        qf[pr][64:128, qsl],
                        start=True, stop=True, tile_position=(64, 0),
                    )
                    pt = ptp.tile([128, 1024], bf16, tag="pt", name="pt")
                    nc.scalar.activation(pt[:], sc[:], Exp, scale=0.125)
                    return pt

                def av_group(st, g, half=None):
                    if g == 0 and half in (None, 0):
                        st["av_e"] = psX.tile([HD + 1, 512], f32, tag="av", name="av_e")
                        st["av_o"] = psX.tile([HD + 1, 512], f32, tag="av", name="av_o")
                    pr = st["pr"]
                    lo = 4 * g if half in (None, 0) else 4 * g + 2
                    hi = 4 * g + 4 if half in (None, 1) else 4 * g + 2
                    for tk in range(lo, hi):
                        nc.tensor.matmul(
                            st["av_e"][:], v_sb[tk][:, 2 * pr, :],
                            st["pts"][tk][:, 0:512],
                            start=(tk == 0), stop=(tk == NT - 1),
                            skip_group_check=True,
                        )
                        nc.tensor.matmul(
                            st["av_o"][:], v_sb[tk][:, 2 * pr + 1, :],
                            st["pts"][tk][:, 512:1024],
                            start=(tk == 0), stop=(tk == NT - 1),
                            skip_group_check=True,
                        )

                def finalize(st, on_act=False):
                    pr, q = st["pr"], st["q"]
                    for e, av in ((0, st["av_e"]), (1, st["av_o"])):
                        hc = 2 * pr + e
                        avs = avsp.tile([HD + 1, 512], f32, tag="avs", name="avs")
                        if on_act:
                            nc.scalar.copy(avs[:], av[:])
                        else:
                            nc.vector.tensor_copy(avs[:], av[:])
                        for j in range(4):
                            qt = q * 4 + j
                            if qt not in out_tiles:
                                out_tiles[qt] = outp.tile(
                                    [128, CPC], f32, tag=f"o{qt}", name=f"o{qt}"
                                )
                            tp = psX.tile([128, HD + 1], f32, tag="av", name="tp")
                            nc.tensor.transpose(
                                tp[:], avs[:, j * 128:(j + 1) * 128],
                                id_sb[0:HD + 1, 0:HD + 1],
                            )
                            rs = rsp.tile([128, 1], f32, tag="rs", name="rs")
                            nc.vector.reciprocal_approx_fast(rs[:], tp[:, HD:HD + 1])
                            nc.vector.tensor_scalar_mul(
                                out_tiles[qt][:, hc * HD:(hc + 1) * HD],
                                tp[:, 0:HD], rs[:],
                            )
                    if pr == 1:
                        for j in range(4):
                            qt = q * 4 + j
                            r0 = qt * 128
                            nc.sync.dma_start(out_d[r0:r0 + 128, :],
                                              out_tiles[qt][:])

                # warmup: PE busy + ACT exp table preload while DMAs land
                wu = stA.tile([128, 512], bf16, tag="qraw", name="wu")
                nc.vector.memset(wu[:], 0.0)
                wups = psX.tile([128, 512], f32, tag="av", name="wups")
                for _ in range(16):
                    nc.tensor.matmul(wups[:], wu[:, 0:128], wu[:],
                                     start=True, stop=True)
                wuexp = stA.tile([128, 512], bf16, tag="rsb", name="wuexp")
                nc.scalar.activation(wuexp[:], wups[:], Exp, scale=0.125)

                K0 = lambda q: (lambda: proj_qk(1, kf[0], 0, q))
                Q0 = lambda q: (lambda: proj_qk(0, qf[0], 0, q))
                K1 = lambda q: (lambda: proj_qk(1, kf[1], 1, q))
                Q1 = lambda q: (lambda: proj_qk(0, qf[1], 1, q))
                V_ = lambda q, t: (lambda: proj_v(q, t))

                HOOKS = {
                    (0, 0): {1: [K0(1), V_(0, 0), V_(0, 1)],
                             2: [K0(2), V_(0, 2), V_(0, 3), V_(1, 0)],
                             3: [K0(3), Q0(1)]},
                    (0, 1): {0: [V_(1, 1), V_(1, 2), V_(1, 3)],
                             1: [V_(2, 0), V_(2, 1), V_(2, 2), V_(2, 3)],
                             2: [V_(3, 0), V_(3, 1), Q0(2)],
                             3: [V_(3, 2), V_(3, 3)]},
                    (0, 2): {0: [Q0(3)], 1: [K1(0)], 2: [K1(1)], 3: [K1(2)]},
                    (0, 3): {0: [K1(3)], 1: [Q1(0)], 2: [Q1(1)], 3: [Q1(2)]},
                    (1, 0): {0: [Q1(3)]},
                }

                chunks = [(0, 0), (0, 1), (0, 2), (0, 3),
                          (1, 0), (1, 1), (1, 2), (1, 3)]
                # first K and Q projections interleaved at kt granularity:
                # kt0-3 of both run while the second DMA halves land
                def first_kq():
                    psk = psX.tile([128, 512], f32, tag="av", name="psk")
                    psq = psX.tile([128, 512], f32, tag="av", name="psq")
                    for kt in range(NKT):
                        nc.tensor.matmul(
                            psk[:], w_sb[1][:, kt, 0:128], xt_sb[0][:, kt, :],
                            start=(kt == 0), stop=(kt == NKT - 1),
                        )
                        nc.tensor.matmul(
                            psq[:], w_sb[0][:, kt, 0:128], xt_sb[0][:, kt, :],
                            start=(kt == 0), stop=(kt == NKT - 1),
                        )
                    for dst, widx, ps in ((kf[0], 1, psk), (qf[0], 0, psq)):
                        qraw = stA.tile([128, 512], bf16, tag="qraw", name="qraw")
                        nc.vector.tensor_scalar(
                            qraw[:], ps[:], bp_sb[:, widx * 2:widx * 2 + 1],
                            None, mybir.AluOpType.add,
                        )
                        nc.tensor.matmul(ps[:], perm_sb[:], qraw[:],
                                         start=True, stop=True)
                        t1 = stA.tile([128, 512], bf16, tag="t1", name="t1")
                        nc.vector.tensor_mul(t1[:], qraw[:], cos_sb[:, 0:512])
                        t2 = stA.tile([128, 512], bf16, tag="t2", name="t2")
                        nc.vector.tensor_mul(t2[:], ps[:], sin_sb[:, 0:512])
                        nc.vector.tensor_add(dst[:, 0:512], t1[:], t2[:])

                first_kq()
                prev = None
                for ci, (pr, q) in enumerate(chunks):
                    last = ci == len(chunks) - 1
                    hooks = HOOKS.get((pr, q), {})
                    pts = []
                    cur = {"pr": pr, "q": q, "pts": pts}
                    for tk in range(NT):
                        g, r = divmod(tk, 4)
                        # hook units spread across the group's tk slots
                        # (r=2 carries the AV groups) so the exp stream
                        # keeps getting fresh score tiles between units
                        hs = hooks.get(g, [])
                        if r == 0:
                            for fn in hs[0:1]:
                                fn()
                        elif r == 1:
                            for fn in hs[1:2]:
                                fn()
                        elif r == 3:
                            for fn in hs[2:]:
                                fn()
                        if r in (2, 3):
                            # AV inputs are long-ready; two-matmul-pair halves
                            # at r2/r3 keep each PE block under the 2-deep exp
                            # backlog so ACT never drains
                            if prev is not None:
                                av_group(prev, g, half=r - 2)
                            if last and g > 0:
                                av_group(cur, g - 1, half=r - 2)
                        pts.append(sc_exp(pr, q, tk))
                    if prev is not None:
                        finalize(prev)
                    prev = cur
                av_group(prev, 3)
                finalize(prev, on_act=True)

    nc.compile()
    return nc


def _get_program():
    if "nc" not in _PROG:
        _PROG["nc"] = _build_program()
    return _PROG["nc"]


def _in_maps(x, wq_w, wq_b, wk_w, wk_b, wv_w, wv_b):
    import ml_dtypes
    bf = ml_dtypes.bfloat16
    cosf, sinf = _rope_tables()
    cosf, sinf = cosf.astype(bf), sinf.astype(bf)
    permT = _perm_matrix().astype(bf)
    ident = np.eye(128, dtype=np.float32)
    maps = []
    for c in range(NCORES):
        b, hg = divmod(c, 4)
        sl = slice(hg * CPC, (hg + 1) * CPC)
        maps.append({
            "xt": np.ascontiguousarray(x[b].T.astype(np.float32)),
            "wqt": np.ascontiguousarray(wq_w[sl].T.astype(np.float32)),
            "wkt": np.ascontiguousarray(wk_w[sl].T.astype(np.float32)),
            "wvt": np.ascontiguousarray(wv_w[sl].T.astype(np.float32)),
            "biasv": np.broadcast_to(
                np.asarray(wv_b[sl], np.float32), (128, CPC)
            ).astype(bf).copy(),
            "biasp": np.stack([
                np.asarray(wq_b[sl][0:128], np.float32),
                np.asarray(wq_b[sl][128:256], np.float32),
                np.asarray(wk_b[sl][0:128], np.float32),
                np.asarray(wk_b[sl][128:256], np.float32),
            ], axis=1),
            "cosf": cosf, "sinf": sinf, "permT": permT, "ident": ident,
        })
    return maps


def _gather(results):
    out = np.empty((B, S, D), dtype=np.float32)
    for c in range(NCORES):
        b, hg = divmod(c, 4)
        out[b, :, hg * CPC:(hg + 1) * CPC] = results[c]["out"]
    return out


def kernel(x, wq_w, wq_b, wk_w, wk_b, wv_w, wv_b):
    from concourse.bass_utils import run_bass_kernel_spmd
    x = np.asarray(x, np.float32)
    wq_w, wq_b = np.asarray(wq_w, np.float32), np.asarray(wq_b, np.float32)
    wk_w, wk_b = np.asarray(wk_w, np.float32), np.asarray(wk_b, np.float32)
    wv_w, wv_b = np.asarray(wv_w, np.float32), np.asarray(wv_b, np.float32)
    nc = _get_program()
    maps = _in_maps(x, wq_w, wq_b, wk_w, wk_b, wv_w, wv_b)
    res = run_bass_kernel_spmd(nc, maps, core_ids=list(range(NCORES)))
    return _gather(res.results)


def kernel_profiled(x, wq_w, wq_b, wk_w, wk_b, wv_w, wv_b):
    """Same as kernel() but requests an NTFF trace; returns (out, results)."""
    from concourse.bass_utils import run_bass_kernel_spmd
    nc = _get_program()
    maps = _in_maps(x, wq_w, wq_b, wk_w, wk_b, wv_w, wv_b)
    res = run_bass_kernel_spmd(
        nc, maps, core_ids=list(range(NCORES)), trace=True
    )
    return _gather(res.results), res
